# revision 23
# baseline (speedup 1.0000x reference)
"""Trainium2 Bass kernel for nn_Encoder_50852412785097 (sparse_attention).

Math (validated against the jax reference to ~1e-6):
  Per (b, h):
    Q = X wQ_h, K = X wK_h, V = X wV_h              (X = inputs[b], [S, D])
    e = (Q K^T) / sqrt(D)
    x = causal_softmax(e)          # == softmax(e) * tril, renormalized
    rr = den - cumsum(exp(e_row))  # den = masked row sum of exp(e)
    decay = exp((theta^2/den) * (t_j - t_i) * rr)   # == exp(-theta^2 (1-c) dt)
    u = exp(e * decay)             # unnormalized second softmax
    out_h = ((u @ V) / sum_j u) @ wO_h
  out[b] = sum_h out_h

Sharding: 16 (b, h) pairs over 8 cores -> core c handles b = c//4,
heads {2*(c%4), 2*(c%4)+1}. Weights replicated; host sums the 4 partial
outputs per batch.

v6 (chunked pipeline, measured-cost-balanced engine assignment):
  - forward scan z = den - cumsum(ex1) (initial=den, op1=subtract), chunked
    left-to-right chained -- matches exp production order; no reverse APs,
    no GPSIMD memset seeds.
  - ex1-exp reads QK scores straight from PSUM (no es dependency).
  - dtt = t_j - t_i materialized once per row-tile (2x tensor_scalar,
    shared by both heads); ww = dtt * z via bf16 tensor_tensor --
    h0 on DVE (2x mode), h1 on GPSIMD.
  - es kept in bf16 in SBUF (copies: DVE for h0, ACT for h1); sarr = es*e2
    bf16 multiply -- h0 on GPSIMD, h1 on DVE (2x).
  - input DMAs on the sync engine; GPSIMD runs only the two TT streams.
  - setup projections copied out of PSUM on DVE (fills the startup bubble
    while ACT begins the main-loop exponentials); work pool 5-deep.
  - scan chunk0 seeded with the partial row-sum denc0 (runs during chunk1's
    exp); the +denc1 shift is folded into chunk0's timestamp stt; rT / t0
    epilogue ops on ACT.
"""

import os
import sys

import numpy as np

B, S, H, D = 2, 2048, 8, 64
P = 128
NT = S // P  # 16 row tiles
NH = 2  # heads per core
NCORES = 8
MASK_VAL = -1e30


def _import_concourse():
    try:
        import concourse.bass  # noqa: F401
    except ImportError:
        for p in ("/opt/trn_rl_repo", "/root/.axon_site/_ro/trn_rl_repo"):
            if os.path.isdir(p) and p not in sys.path:
                sys.path.insert(0, p)
        import concourse.bass  # noqa: F401


def build_nc(ww_stt=False, es_f32=False):
    """Build the SPMD single-core program (same on all 8 cores).

    ww_stt: fall back to the one-op scalar_tensor_tensor for ww (no dtt).
    es_f32: keep the SBUF es copy in f32 (precision fallback; disables the
            DVE-side 2x multiply).
    """
    _import_concourse()
    import concourse.bass as bass
    import concourse.bacc as bacc
    from concourse import mybir
    from concourse.tile import TileContext

    f32 = mybir.dt.float32
    bf16 = mybir.dt.bfloat16
    Alu = mybir.AluOpType
    Act = mybir.ActivationFunctionType

    esdt = f32 if es_f32 else bf16

    nc = bacc.Bacc("TRN2", target_bir_lowering=False, debug=False)

    # --- external I/O (per core) ---
    xT_h = nc.dram_tensor("xT", [D, S], f32, kind="ExternalInput")     # X^T
    tsj_h = nc.dram_tensor("tsj", [1, S], f32, kind="ExternalInput")   # t_j row
    tsi_h = nc.dram_tensor("tsi", [P, NT], f32, kind="ExternalInput")  # t_i cols
    wq_h = nc.dram_tensor("wq", [D, NH * D], f32, kind="ExternalInput")
    wk_h = nc.dram_tensor("wk", [D, NH * D], f32, kind="ExternalInput")
    wv_h = nc.dram_tensor("wv", [D, NH * D], f32, kind="ExternalInput")
    wo_h = nc.dram_tensor("wo", [D, NH * D], f32, kind="ExternalInput")
    th_h = nc.dram_tensor("th", [1, 1], f32, kind="ExternalInput")
    y_h = nc.dram_tensor("y", [S, D], f32, kind="ExternalOutput")

    # --- NEFF-embedded constants ---
    mask_np = np.triu(np.ones((P, P), np.float32), k=1) * np.float32(MASK_VAL)
    mask_dram = nc.inline_tensor(mask_np, name="maskc")
    ident_dram = nc.inline_tensor(np.eye(P, dtype=np.float32), name="identc")

    with TileContext(nc) as tc:
        from contextlib import ExitStack

        with ExitStack() as ctx:
            consts = ctx.enter_context(tc.tile_pool(name="consts", bufs=1))

            def load(shape, handle_ap, via, name, dt=f32):
                stage = consts.tile(shape, f32, tag=f"stg_{name}")
                nc.sync.dma_start(out=stage, in_=handle_ap)
                dst = consts.tile(shape, dt, tag=name)
                via(dst, stage)
                return dst

            # PE-consumed: staged via DVE
            mask = load([P, P], mask_dram[:, :], nc.vector.tensor_copy, "mask")
            identb = load([P, P], ident_dram[:, :], nc.vector.tensor_copy,
                          "identb", dt=bf16)
            xT = load([D, S], xT_h[:, :], nc.vector.tensor_copy, "xT")
            xTb = consts.tile([D, S], bf16, tag="xTb")
            nc.vector.tensor_copy(xTb, xT)
            wq = load([D, NH * D], wq_h[:, :], nc.vector.tensor_copy, "wq", dt=bf16)
            wk = load([D, NH * D], wk_h[:, :], nc.vector.tensor_copy, "wk", dt=bf16)
            wv = load([D, NH * D], wv_h[:, :], nc.vector.tensor_copy, "wv", dt=bf16)
            wo = load([D, NH * D], wo_h[:, :], nc.vector.tensor_copy, "wo", dt=bf16)

            # DVE-consumed (multi-DMA): staged via ACT
            tsj_ap = tsj_h[:, :]
            tsj_b = bass.AP(
                tensor=tsj_ap.tensor, offset=tsj_ap.offset,
                ap=[[0, P], list(tsj_ap.ap[-1])],
            )
            tsj = load([P, S], tsj_b, nc.scalar.copy, "tsj")
            tsi = load([P, NT], tsi_h[:, :], nc.scalar.copy, "tsi")

            # theta broadcast -> th2 = theta^2
            thb = consts.tile([P, 1], f32)
            th_ap = th_h[:, :]
            th_b = bass.AP(
                tensor=th_ap.tensor, offset=th_ap.offset,
                ap=[[0, P], list(th_ap.ap[-1])],
            )
            nc.sync.dma_start(out=thb, in_=th_b)
            th2 = consts.tile([P, 1], f32)
            nc.vector.tensor_mul(th2, thb, thb)

            ones = consts.tile([P, S], bf16)
            nc.vector.memset(ones, 1.0)

            # --- projections: qt (scaled by 1/8), kt: [64, NH*S]; v: [128, NH*NT*64] ---
            qt = consts.tile([D, NH * S], bf16)
            kt = consts.tile([D, NH * S], bf16)
            v = consts.tile([P, NH * NT * D], bf16)
            with tc.tile_pool(name="psetup", bufs=2, space="PSUM") as psetup:
                for h in range(NH):
                    for sc in range(S // 512):
                        pq = psetup.tile([D, 512], f32, tag="pq")
                        nc.tensor.matmul(
                            pq, lhsT=wq[:, h * D:(h + 1) * D],
                            rhs=xTb[:, 512 * sc:512 * (sc + 1)],
                            start=True, stop=True,
                        )
                        nc.vector.tensor_scalar(
                            qt[:, h * S + 512 * sc: h * S + 512 * (sc + 1)],
                            pq, scalar1=0.125, scalar2=None, op0=Alu.mult)
                        pk = psetup.tile([D, 512], f32, tag="pk")
                        nc.tensor.matmul(
                            pk, lhsT=wk[:, h * D:(h + 1) * D],
                            rhs=xTb[:, 512 * sc:512 * (sc + 1)],
                            start=True, stop=True,
                        )
                        nc.vector.tensor_copy(kt[:, h * S + 512 * sc: h * S + 512 * (sc + 1)], pk)
                    for st in range(NT):
                        pv = psetup.tile([P, D], f32, tag="pv")
                        nc.tensor.matmul(
                            pv, lhsT=xTb[:, P * st:P * (st + 1)],
                            rhs=wv[:, h * D:(h + 1) * D],
                            start=True, stop=True,
                        )
                        nc.vector.tensor_copy(
                            v[:, (h * NT + st) * D:(h * NT + st + 1) * D], pv)

            # --- main pipeline ---
            work = ctx.enter_context(tc.tile_pool(name="work", bufs=5))
            dpool = ctx.enter_context(tc.tile_pool(name="dpool", bufs=2))
            small = ctx.enter_context(tc.tile_pool(name="small", bufs=6))
            ppe = ctx.enter_context(tc.tile_pool(name="ppe", bufs=2,
                                                 space="PSUM"))
            ppt = ctx.enter_context(
                tc.tile_pool(name="ppt", bufs=2, space="PSUM"))
            pprT = ctx.enter_context(tc.tile_pool(name="pprT", bufs=1, space="PSUM"))
            ppo = ctx.enter_context(tc.tile_pool(name="ppo", bufs=1, space="PSUM"))

            CH = 1024  # pipeline chunk (columns)

            for ti in range(NT):
                W = P * (ti + 1)
                nch = (W + CH - 1) // CH
                t_i = tsi[:, ti:ti + 1]
                po2 = ppo.tile([P, NH, D + 1], f32)
                den2_h = []

                if not ww_stt:
                    # dtt = t_j - t_i, shared by both heads (2x tensor_scalar)
                    dtt = dpool.tile([P, W], bf16, tag="dtt")
                    nc.vector.tensor_scalar(dtt, tsj[:, :W], scalar1=t_i,
                                            scalar2=None, op0=Alu.subtract)

                for h in range(NH):
                    # scores in PSUM chunks; ex1-exp reads PSUM directly,
                    # es copied to SBUF (bf16) for the sarr multiply.
                    cpy = nc.vector.tensor_copy if h == 0 else nc.scalar.copy
                    es = work.tile([P, S], esdt, tag="es")
                    ex1 = work.tile([P, S], bf16, tag="ex1")
                    denc = small.tile([P, nch], f32, tag="denc")
                    qrow = qt[:, h * S + P * ti: h * S + P * (ti + 1)]
                    for c in range(nch):
                        lo, hi = CH * c, min(W, CH * (c + 1))
                        pe = ppe.tile([P, CH], f32, tag="pe")
                        j0 = lo
                        while j0 < hi:
                            j1 = min(hi, j0 + 512)
                            nc.tensor.matmul(
                                pe[:, j0 - lo:j1 - lo], lhsT=qrow,
                                rhs=kt[:, h * S + j0: h * S + j1],
                                start=True, stop=True,
                            )
                            j0 = j1
                        cols = hi - lo
                        if hi == W:
                            # causal mask applied in PSUM so the PSUM-read
                            # exp sees masked scores; SBUF copy follows
                            nc.vector.tensor_add(
                                pe[:, cols - P:cols], pe[:, cols - P:cols],
                                mask)
                            cpy(es[:, lo:hi], pe[:, :cols])
                        else:
                            cpy(es[:, lo:hi], pe[:, :cols])
                        nc.scalar.activation(ex1[:, lo:hi], pe[:, :cols],
                                             Act.Exp,
                                             accum_out=denc[:, c:c + 1])
                    # den = sum of chunk sums; spp = theta^2 / den
                    if nch > 1:
                        den = small.tile([P, 1], f32, tag="den")
                        nc.vector.tensor_reduce(den, denc, mybir.AxisListType.X,
                                                Alu.add)
                    else:
                        den = denc[:, 0:1]
                    rden = small.tile([P, 1], f32, tag="rden")
                    nc.vector.reciprocal(rden, den)
                    spp = small.tile([P, 1], f32, tag="spp")
                    nc.vector.tensor_mul(spp, th2, rden)

                    # forward scan, chunk0 seeded with the partial den
                    # (denc0) so it can run during chunk1's exp; chunk1 is
                    # re-seeded with the corrected boundary value.
                    z = work.tile([P, S], bf16, tag="z")
                    for c in range(nch):
                        lo, hi = CH * c, min(W, CH * (c + 1))
                        if c == 0:
                            init = denc[:, 0:1]
                        else:
                            zc = small.tile([P, 1], f32, tag="zc")
                            nc.vector.tensor_scalar(
                                zc, z[:, lo - 1:lo], scalar1=denc[:, 1:2],
                                scalar2=None, op0=Alu.add)
                            init = zc
                        nc.vector.tensor_tensor_scan(
                            z[:, lo:hi], ones[:, :hi - lo], ex1[:, lo:hi],
                            initial=init, op0=Alu.mult, op1=Alu.subtract,
                        )
                    # ww = dtt*z (h0: DVE 2x TT, h1: GPSIMD); e2 = exp(spp*ww)
                    # sarr = es*e2 (h0: GPSIMD, h1: DVE 2x); u = exp(sarr)
                    e2 = work.tile([P, S], bf16, tag="e2")
                    u = work.tile([P, S], bf16, tag="u")
                    den2c = small.tile([P, nch], f32, tag="den2c")
                    ww_eng = nc.vector if h == 0 else nc.gpsimd
                    mul_eng = nc.gpsimd if h == 0 else nc.vector
                    for c in range(nch):
                        lo, hi = CH * c, min(W, CH * (c + 1))
                        if nch > 1 and c == 0:
                            # z chunk0 is denc0-cs; fold the +denc1 shift into
                            # the timestamp multiply (stt is DVE-only)
                            nc.vector.scalar_tensor_tensor(
                                z[:, lo:hi], in0=z[:, lo:hi],
                                scalar=denc[:, 1:2], in1=dtt[:, lo:hi],
                                op0=Alu.add, op1=Alu.mult,
                            )
                        else:
                            ww_eng.tensor_mul(z[:, lo:hi], z[:, lo:hi],
                                              dtt[:, lo:hi])
                        nc.scalar.activation(e2[:, lo:hi], z[:, lo:hi],
                                             Act.Exp, scale=spp)
                        mul_eng.tensor_mul(e2[:, lo:hi], es[:, lo:hi],
                                           e2[:, lo:hi])
                        nc.scalar.activation(u[:, lo:hi], e2[:, lo:hi],
                                             Act.Exp,
                                             accum_out=den2c[:, c:c + 1])
                    if nch > 1:
                        den2 = small.tile([P, 1], f32, tag="den2")
                        nc.vector.tensor_reduce(den2, den2c,
                                                mybir.AxisListType.X, Alu.add)
                    else:
                        den2 = den2c[:, 0:1]
                    rden2 = small.tile([P, 1], f32, tag="rden2")
                    nc.vector.reciprocal(rden2, den2)
                    den2_h.append(rden2)

                    # AV: retT[e, i] = sum_j v[j, e] u[i, j]; transposes in
                    # groups of 8 blocks -> one 1024-wide PSUM tile -> 1 copy
                    prT = pprT.tile([D, P], f32, tag="prT")
                    njb = ti + 1
                    for g0 in range(0, njb, 8):
                        gn = min(8, njb - g0)
                        uT8 = small.tile([P, 8 * P], bf16, tag="uT8")
                        pt = ppt.tile([P, 8 * P], bf16, tag="pt")
                        for q in range(gn):
                            nc.tensor.transpose(
                                pt[:, q * P:(q + 1) * P],
                                u[:, (g0 + q) * P:(g0 + q + 1) * P], identb)
                        nc.vector.tensor_copy(uT8[:, :gn * P], pt[:, :gn * P])
                        for q in range(gn):
                            jb = g0 + q
                            nc.tensor.matmul(
                                prT, lhsT=v[:, (h * NT + jb) * D:(h * NT + jb + 1) * D],
                                rhs=uT8[:, q * P:(q + 1) * P],
                                start=(jb == 0), stop=(jb == ti),
                            )
                    rT = small.tile([D, P], bf16, tag="rT")
                    nc.scalar.copy(rT, prT[0:D, :])
                    nc.tensor.matmul(po2[:, h, 0:D], lhsT=rT,
                                     rhs=wo[:, h * D:(h + 1) * D],
                                     start=True, stop=True)

                # y = po2[0]/den2_0 + po2[1]/den2_1 ; DMA out
                t0 = small.tile([P, D], f32, tag="t0")
                nc.scalar.mul(t0, po2[:, 0, :D], den2_h[0])
                ys = small.tile([P, D], f32, tag="ys")
                nc.vector.scalar_tensor_tensor(
                    ys, in0=po2[:, 1, :D], scalar=den2_h[1], in1=t0,
                    op0=Alu.mult, op1=Alu.add,
                )
                nc.sync.dma_start(out=y_h[P * ti:P * (ti + 1), :], in_=ys)

    if not nc.is_finalized():
        nc.finalize()
    return nc


_NC_CACHE = {}

KERNEL_FLAGS = {}


def _get_nc():
    key = tuple(sorted(KERNEL_FLAGS.items()))
    if key not in _NC_CACHE:
        _NC_CACHE[key] = build_nc(**KERNEL_FLAGS)
    return _NC_CACHE[key]


def make_in_maps(inputs, timestamp, wQ, wK, wV, wO, theta):
    x = np.asarray(inputs, np.float32)
    t = np.asarray(timestamp).astype(np.float32)
    wQ = np.asarray(wQ, np.float32)
    wK = np.asarray(wK, np.float32)
    wV = np.asarray(wV, np.float32)
    wO = np.asarray(wO, np.float32)
    theta = np.asarray(theta, np.float32)

    in_maps = []
    for c in range(NCORES):
        b = c // 4
        h0 = NH * (c % 4)
        in_maps.append({
            "xT": np.ascontiguousarray(x[b].T),
            "tsj": np.ascontiguousarray(t[b][None, :]),
            "tsi": np.ascontiguousarray(t[b].reshape(NT, P).T),
            "wq": np.ascontiguousarray(np.concatenate([wQ[h0], wQ[h0 + 1]], axis=1)),
            "wk": np.ascontiguousarray(np.concatenate([wK[h0], wK[h0 + 1]], axis=1)),
            "wv": np.ascontiguousarray(np.concatenate([wV[h0], wV[h0 + 1]], axis=1)),
            "wo": np.ascontiguousarray(np.concatenate(
                [wO[h0 * D:(h0 + 1) * D], wO[(h0 + 1) * D:(h0 + 2) * D]], axis=1)),
            "th": np.ascontiguousarray(theta.reshape(1, 1)),
        })
    return in_maps


def kernel(inputs, timestamp, wQ, wK, wV, wO, theta, _trace=False, _trace_kwargs=None):
    _import_concourse()
    from concourse.bass_utils import run_bass_kernel_spmd

    nc = _get_nc()
    in_maps = make_in_maps(inputs, timestamp, wQ, wK, wV, wO, theta)
    res = run_bass_kernel_spmd(
        nc, in_maps, list(range(NCORES)),
        trace=_trace, **(_trace_kwargs or {}),
    )
    out = np.zeros((B, S, D), np.float32)
    for c in range(NCORES):
        out[c // 4] += res.results[c]["y"]
    if _trace:
        return out, res
    return out


if __name__ == "__main__":
    nc = build_nc()
    print("built ok")
